# revision 17
# baseline (speedup 1.0000x reference)
"""Trainium2 Bass kernel for CellPathwayPoolingAggregator (segment mean).

out[b, p] = (1/segment_sizes[p]) * sum_{k: segment_ids[k]==p} x[b, flat_indices[k]]

Strategy (8 cores, sharded by contiguous pathway ranges):
  - Host: split the 1000 pathways into 8 contiguous ranges (<=128 pathways
    each) balancing per-core unique-gene counts. For each core, dedupe its
    gene rows and pack them into a contiguous DRAM slab in float8_e3m4
    (T k-tiles of 128 gene rows, grouped in GRP-tile DMA groups laid out so
    each DMA is a perfect 128-partition x (GRP*2KB)-per-partition contiguous
    transfer). A per-core count matrix S (e3m4, exact for small counts)
    carries the (gene, pathway) multiplicities.
  - Device (per core): plain sequential dma_start per group pulls the slab
    into SBUF; a PE matmul per (k-tile, 512-batch bank) accumulates
    pathway x batch sums into 4 PSUM banks (S tile stationary, gathered
    rows moving, fp32 accumulate). DVE/ACT scale rows by 1/segment_sizes,
    DMA stores the (128, 2048) f32 output slice; host reassembles.

e3m4 quantization of x gives rel err ~1.3e-2 (< 2e-2 tolerance); counts and
1/size scaling stay exact (counts are small ints; scale applied in f32).
"""

import sys

import numpy as np
import ml_dtypes

_TRN_REPO = "/opt/trn_rl_repo"
if _TRN_REPO not in sys.path:
    sys.path.insert(0, _TRN_REPO)

import concourse.bass as bass  # noqa: F401
import concourse.mybir as mybir
import concourse.tile as tile
from concourse import bacc
from concourse.bass_utils import run_bass_kernel_spmd

B, G, P = 2048, 10000, 1000
NCORES = 8
PC = 128          # max pathways per core (psum partition dim)
NB = B // 512     # matmul N-slices per K-tile (4 banks of 512 f32)
NWARM = 12        # PE warmup matmuls (ramp the tensor engine p-state)

F8 = ml_dtypes.float8_e3m4


def _group_sizes(T):
    """k-tiles per DMA group: 1-tile leading group (lands early, gates the
    first matmul), then uniform 2-tile (4KB/partition) groups."""
    gs = [1]
    while sum(gs) < T - 1:
        gs.append(2)
    gs[-1] -= sum(gs) - T
    return [g for g in gs if g > 0]


def _split_ranges(seg_sorted, idx_sorted):
    """Contiguous pathway ranges, <=128 pathways each, minimizing the max
    per-core count of UNIQUE genes (which sets T and hence DMA/PE work)."""
    seg_starts = np.searchsorted(seg_sorted, np.arange(P + 1), side="left")

    def feasible(U):
        bounds = [0]
        for c in range(NCORES):
            lo_p = bounds[-1]
            if lo_p >= P:
                return None
            best = lo_p + 1
            hi_cap = min(P, lo_p + PC)
            lo_e = seg_starts[lo_p]
            for hi_p in range(lo_p + 1, hi_cap + 1):
                nu = len(np.unique(idx_sorted[lo_e : seg_starts[hi_p]]))
                if nu <= U:
                    best = hi_p
                else:
                    break
            bounds.append(best)
        return bounds if bounds[-1] >= P else None

    lo_t, hi_t = 1, (len(idx_sorted) + 127) // 128 + 1
    best_bounds = None
    while lo_t <= hi_t:
        mid = (lo_t + hi_t) // 2
        b = feasible(mid * 128)
        if b is not None:
            best_bounds = b
            hi_t = mid - 1
        else:
            lo_t = mid + 1
    if best_bounds is None:
        best_bounds = list(
            np.minimum(np.arange(NCORES + 1) * ((P + NCORES - 1) // NCORES), P)
        )
    best_bounds[-1] = P
    return best_bounds


def _build_schedule(flat_indices, segment_ids):
    seg = np.asarray(segment_ids, dtype=np.int64)
    idx = np.asarray(flat_indices, dtype=np.int64)
    order = np.argsort(seg, kind="stable")
    seg = seg[order]
    idx = idx[order]

    bounds = _split_ranges(seg, idx)
    cores = []
    for c in range(NCORES):
        lo_p, hi_p = bounds[c], bounds[c + 1]
        lo = np.searchsorted(seg, lo_p, side="left")
        hi = np.searchsorted(seg, hi_p, side="left")
        uidx, inv = np.unique(idx[lo:hi], return_inverse=True)
        cores.append((lo_p, hi_p, uidx, inv, seg[lo:hi] - lo_p))

    T = max(1, max((len(u) + 127) // 128 for _, _, u, _, _ in cores))
    Kpad = T * 128

    s_sbs, uidx_pads = [], []
    for lo_p, hi_p, uidx, inv, cols in cores:
        nu = len(uidx)
        # padded unique-gene list; pad rows point at gene 0 but S is zero there
        uidx_pad = np.concatenate([uidx, np.zeros(Kpad - nu, np.int64)])
        S = np.zeros((Kpad, PC), np.float32)
        np.add.at(S, (inv, cols), 1.0)
        S = S.astype(F8)
        s_sbs.append(
            np.ascontiguousarray(
                S.reshape(T, 128, PC).transpose(1, 0, 2).reshape(128, -1)
            )
        )
        uidx_pads.append(uidx_pad)
    return bounds, uidx_pads, s_sbs, T


def _build_program(T):
    nc = bacc.Bacc(
        "TRN2",
        target_bir_lowering=False,
        debug=False,
        num_devices=NCORES,
        num_swdge_queues=1,
    )
    f8, f32, bf16 = mybir.dt.float8e3, mybir.dt.float32, mybir.dt.bfloat16

    gsz = _group_sizes(T)
    NG = len(gsz)
    slab_ds = [
        nc.dram_tensor(f"slab{g}", [128, gsz[g] * B], f8, kind="ExternalInput")
        for g in range(NG)
    ]
    s_d = nc.dram_tensor("smat", [128, T * PC], f8, kind="ExternalInput")
    inv_d = nc.dram_tensor("invsz", [128, 1], f32, kind="ExternalInput")
    out_d = nc.dram_tensor("out", [PC, B], bf16, kind="ExternalOutput")

    with tile.TileContext(nc) as tc:
        with (
            tc.tile_pool(name="sb", bufs=1) as pool,
            tc.tile_pool(name="psum", bufs=1, space="PSUM") as ppool,
        ):
            # Warmup source: memset on the (otherwise idle) Vector engine —
            # no DMA dependency, so the tensor engine starts ramping its
            # p-state immediately.
            wsrc = pool.tile([128, 512], f8, tag="wsrc")
            nc.vector.memset(wsrc[:], 0)

            # Head scheduling: the 1-tile group 0 rides alone at the front
            # of the Scalar HWDGE queue (its completion semaphore gates the
            # first matmul); smat leads the Sync queue; the remaining groups
            # follow on Sync. invsz (needed only at eviction) trails.
            s_sb = pool.tile([128, T * PC], f8, tag="smat")
            nc.sync.dma_start(s_sb[:], s_d.ap())

            psb = [
                ppool.tile([128, 512], f32, tag=f"ps{n}", name=f"ps{n}")
                for n in range(NB)
            ]
            wps = ppool.tile([128, 512], f32, tag="pswarm", name="pswarm")

            gts = []
            for g in range(NG):
                gt = pool.tile([128, gsz[g] * B], f8, tag=f"gt{g}")
                eng = nc.scalar if g == 0 else nc.sync
                eng.dma_start(gt[:], slab_ds[g].ap())
                gts.append(gt)

            inv_sb = pool.tile([128, 1], f32, tag="invsz")
            nc.scalar.dma_start(inv_sb[:], inv_d.ap())

            # Warmup matmuls: ramp the PE p-state while the first slab
            # group + smat are still in flight.
            for _ in range(NWARM):
                nc.tensor.matmul(
                    wps[:], wsrc[:, :128], wsrc[:], start=True, stop=True
                )

            tstarts = np.cumsum([0] + gsz)
            for g in range(NG):
                gt = gts[g]
                for cc in range(gsz[g]):
                    t = int(tstarts[g]) + cc
                    for n in range(NB):
                        nc.tensor.matmul(
                            psb[n][:],
                            s_sb[:, t * PC : (t + 1) * PC],
                            gt[:, cc * B + n * 512 : cc * B + (n + 1) * 512],
                            start=(t == 0),
                            stop=(t == T - 1),
                        )

            # Eviction: DVE and ACT alternate per bank into two bf16 tiles,
            # each stored with one DMA (2KB/partition transfers).
            ots = [
                pool.tile([128, 1024], bf16, tag=f"ot{i}", name=f"ot{i}")
                for i in range(2)
            ]
            for n in range(NB):
                ot = ots[n // 2][:, (n % 2) * 512 : (n % 2 + 1) * 512]
                if n % 2 == 0:
                    nc.vector.tensor_scalar_mul(ot, psb[n][:], inv_sb[:])
                else:
                    nc.scalar.activation(
                        ot,
                        psb[n][:],
                        mybir.ActivationFunctionType.Identity,
                        scale=inv_sb[:],
                    )
                if n % 2 == 1:
                    nc.sync.dma_start(
                        out_d.ap()[:, (n // 2) * 1024 : (n // 2 + 1) * 1024],
                        ots[n // 2][:],
                    )
    return nc


def _prepare(gene_set_features, flat_indices, segment_ids, segment_sizes):
    bounds, uidx_pads, s_sbs, T = _build_schedule(flat_indices, segment_ids)
    nc = _build_program(T)
    nc.compile()

    x = np.asarray(gene_set_features, dtype=np.float32)
    xt8 = np.ascontiguousarray(x.T).astype(F8)  # (G, B) e3m4
    sizes = np.asarray(segment_sizes, dtype=np.float32)
    gsz = _group_sizes(T)
    tstarts = np.cumsum([0] + gsz)

    in_maps = []
    for c in range(NCORES):
        lo_p, hi_p = bounds[c], bounds[c + 1]
        inv = np.ones((128, 1), np.float32)
        inv[: hi_p - lo_p, 0] = 1.0 / sizes[lo_p:hi_p]
        m = {"invsz": inv, "smat": s_sbs[c]}
        up = uidx_pads[c].reshape(T, 128)  # [t, p]
        for g, gs in enumerate(gsz):
            # slab row p holds the gs gene rows for partition p of group g,
            # concatenated: [ktile tstarts[g]+cc, partition p] for cc in gs.
            tiles = up[tstarts[g] : tstarts[g] + gs]        # [gs, 128]
            perm = tiles.T.reshape(-1)                       # [p, cc]
            m[f"slab{g}"] = np.ascontiguousarray(
                xt8[perm].reshape(128, gs * B)
            )
        in_maps.append(m)
    return nc, in_maps, bounds


def kernel(gene_set_features, flat_indices, segment_ids, segment_sizes, _res_hook=None):
    nc, in_maps, bounds = _prepare(
        gene_set_features, flat_indices, segment_ids, segment_sizes
    )
    res = run_bass_kernel_spmd(nc, in_maps, list(range(NCORES)))
    if _res_hook is not None:
        _res_hook(res)
    outT = np.empty((P, B), np.float32)
    for c in range(NCORES):
        lo_p, hi_p = bounds[c], bounds[c + 1]
        outT[lo_p:hi_p] = np.asarray(res.results[c]["out"]).astype(np.float32)[
            : hi_p - lo_p
        ]
    return np.ascontiguousarray(outT.T)


# revision 21
# speedup vs baseline: 1.1509x; 1.1509x over previous
"""Trainium2 Bass kernel for CellPathwayPoolingAggregator (segment mean).

out[b, p] = (1/segment_sizes[p]) * sum_{k: segment_ids[k]==p} x[b, flat_indices[k]]

Strategy (8 cores, sharded by contiguous pathway ranges):
  - Host: split the 1000 pathways into 8 contiguous ranges (<=128 pathways
    each) balancing per-core unique-gene counts. For each core, dedupe its
    gene rows and pack them into a contiguous DRAM slab in float8_e3m4
    (T k-tiles of 128 gene rows, grouped in GRP-tile DMA groups laid out so
    each DMA is a perfect 128-partition x (GRP*2KB)-per-partition contiguous
    transfer). A per-core count matrix S (e3m4, exact for small counts)
    carries the (gene, pathway) multiplicities.
  - Device (per core): plain sequential dma_start per group pulls the slab
    into SBUF; a PE matmul per (k-tile, 512-batch bank) accumulates
    pathway x batch sums into 4 PSUM banks (S tile stationary, gathered
    rows moving, fp32 accumulate). DVE/ACT scale rows by 1/segment_sizes,
    DMA stores the (128, 2048) f32 output slice; host reassembles.

e3m4 quantization of x gives rel err ~1.3e-2 (< 2e-2 tolerance); counts and
1/size scaling stay exact (counts are small ints; scale applied in f32).
"""

import sys

import numpy as np
import ml_dtypes

_TRN_REPO = "/opt/trn_rl_repo"
if _TRN_REPO not in sys.path:
    sys.path.insert(0, _TRN_REPO)

import concourse.bass as bass  # noqa: F401
import concourse.mybir as mybir
import concourse.tile as tile
from concourse import bacc
from concourse.bass_utils import run_bass_kernel_spmd

B, G, P = 2048, 10000, 1000
NCORES = 8
PC = 128          # max pathways per core (psum partition dim)
NB = B // 512     # matmul N-slices per K-tile (4 banks of 512 f32)
NWARM = 12        # PE warmup matmuls (ramp the tensor engine p-state)

F8 = ml_dtypes.float8_e3m4


def _group_sizes(T):
    """k-tiles per DMA group: uniform 2-tile (4KB/partition) groups.
    (A smaller leading group starves the PE at t=1 and resets its p-state
    ramp — measured worse.)"""
    gs = [2] * (T // 2)
    if T % 2:
        gs.append(1)
    return gs


def _split_ranges(seg_sorted, idx_sorted):
    """Contiguous pathway ranges, <=128 pathways each, minimizing the max
    per-core count of UNIQUE genes (which sets T and hence DMA/PE work)."""
    seg_starts = np.searchsorted(seg_sorted, np.arange(P + 1), side="left")

    def feasible(U):
        bounds = [0]
        for c in range(NCORES):
            lo_p = bounds[-1]
            if lo_p >= P:
                return None
            best = lo_p + 1
            hi_cap = min(P, lo_p + PC)
            lo_e = seg_starts[lo_p]
            for hi_p in range(lo_p + 1, hi_cap + 1):
                nu = len(np.unique(idx_sorted[lo_e : seg_starts[hi_p]]))
                if nu <= U:
                    best = hi_p
                else:
                    break
            bounds.append(best)
        return bounds if bounds[-1] >= P else None

    lo_t, hi_t = 1, (len(idx_sorted) + 127) // 128 + 1
    best_bounds = None
    while lo_t <= hi_t:
        mid = (lo_t + hi_t) // 2
        b = feasible(mid * 128)
        if b is not None:
            best_bounds = b
            hi_t = mid - 1
        else:
            lo_t = mid + 1
    if best_bounds is None:
        best_bounds = list(
            np.minimum(np.arange(NCORES + 1) * ((P + NCORES - 1) // NCORES), P)
        )
    best_bounds[-1] = P
    return best_bounds


def _build_schedule(flat_indices, segment_ids):
    seg = np.asarray(segment_ids, dtype=np.int64)
    idx = np.asarray(flat_indices, dtype=np.int64)
    order = np.argsort(seg, kind="stable")
    seg = seg[order]
    idx = idx[order]

    bounds = _split_ranges(seg, idx)
    cores = []
    for c in range(NCORES):
        lo_p, hi_p = bounds[c], bounds[c + 1]
        lo = np.searchsorted(seg, lo_p, side="left")
        hi = np.searchsorted(seg, hi_p, side="left")
        uidx, inv = np.unique(idx[lo:hi], return_inverse=True)
        cores.append((lo_p, hi_p, uidx, inv, seg[lo:hi] - lo_p))

    T = max(1, max((len(u) + 127) // 128 for _, _, u, _, _ in cores))
    T += T % 2  # even T: uniform 2-tile DMA groups (single-tag tile pool)
    Kpad = T * 128

    s_sbs, uidx_pads = [], []
    for lo_p, hi_p, uidx, inv, cols in cores:
        nu = len(uidx)
        # padded unique-gene list; pad rows point at gene 0 but S is zero there
        uidx_pad = np.concatenate([uidx, np.zeros(Kpad - nu, np.int64)])
        S = np.zeros((Kpad, PC), np.float32)
        np.add.at(S, (inv, cols), 1.0)
        S = S.astype(F8)
        s_sbs.append(
            np.ascontiguousarray(
                S.reshape(T, 128, PC).transpose(1, 0, 2).reshape(128, -1)
            )
        )
        uidx_pads.append(uidx_pad)
    return bounds, uidx_pads, s_sbs, T


def _build_program(T):
    nc = bacc.Bacc(
        "TRN2",
        target_bir_lowering=False,
        debug=False,
        num_devices=NCORES,
        num_swdge_queues=1,
    )
    f8, f32, bf16 = mybir.dt.float8e3, mybir.dt.float32, mybir.dt.bfloat16

    gsz = _group_sizes(T)
    NG = len(gsz)
    slab_ds = [
        nc.dram_tensor(f"slab{g}", [128, gsz[g] * B], f8, kind="ExternalInput")
        for g in range(NG)
    ]
    s_d = nc.dram_tensor("smat", [128, T * PC], f8, kind="ExternalInput")
    inv_d = nc.dram_tensor("invsz", [128, 1], f32, kind="ExternalInput")
    out_d = nc.dram_tensor("out", [PC, B], bf16, kind="ExternalOutput")

    with tile.TileContext(nc) as tc:
        with (
            tc.tile_pool(name="sb", bufs=1) as pool,
            tc.tile_pool(name="slabp", bufs=NG) as gpool,
            tc.tile_pool(name="psum", bufs=1, space="PSUM") as ppool,
        ):
            # Warmup source: memset on the (otherwise idle) Vector engine —
            # no DMA dependency, so the tensor engine starts ramping its
            # p-state immediately.
            wsrc = pool.tile([128, 512], f8, tag="wsrc")
            nc.vector.memset(wsrc[:], 0)

            # smat/invsz on the Scalar HWDGE queue, in parallel with the
            # slab groups on Sync. (GpSimd DMA is SWDGE — too slow here.)
            s_sb = pool.tile([128, T * PC], f8, tag="smat")
            nc.scalar.dma_start(s_sb[:], s_d.ap())
            inv_sb = pool.tile([128, 1], f32, tag="invsz")
            nc.scalar.dma_start(inv_sb[:], inv_d.ap())

            psb = [
                ppool.tile([128, 512], f32, tag=f"ps{n}", name=f"ps{n}")
                for n in range(NB)
            ]
            wps = ppool.tile([128, 512], f32, tag="pswarm", name="pswarm")

            gts = []
            for g in range(NG):
                gt = gpool.tile([128, gsz[g] * B], f8, tag="gt")
                nc.sync.dma_start(gt[:], slab_ds[g].ap())
                gts.append(gt)

            # Warmup matmuls: ramp the PE p-state while the first slab
            # group + smat are still in flight.
            for _ in range(NWARM):
                nc.tensor.matmul(
                    wps[:], wsrc[:, :128], wsrc[:], start=True, stop=True
                )

            tstarts = np.cumsum([0] + gsz)
            for g in range(NG):
                gt = gts[g]
                for cc in range(gsz[g]):
                    t = int(tstarts[g]) + cc
                    for n in range(NB):
                        nc.tensor.matmul(
                            psb[n][:],
                            s_sb[:, t * PC : (t + 1) * PC],
                            gt[:, cc * B + n * 512 : cc * B + (n + 1) * 512],
                            start=(t == 0),
                            stop=(t == T - 1),
                        )

            # Eviction: DVE and ACT alternate per bank into two bf16 tiles,
            # each stored with one DMA (2KB/partition transfers).
            ots = [
                pool.tile([128, 1024], bf16, tag=f"ot{i}", name=f"ot{i}")
                for i in range(2)
            ]
            for n in range(NB):
                ot = ots[n // 2][:, (n % 2) * 512 : (n % 2 + 1) * 512]
                if n % 2 == 0:
                    nc.vector.tensor_scalar_mul(ot, psb[n][:], inv_sb[:])
                else:
                    nc.scalar.activation(
                        ot,
                        psb[n][:],
                        mybir.ActivationFunctionType.Identity,
                        scale=inv_sb[:],
                    )
                if n % 2 == 1:
                    nc.sync.dma_start(
                        out_d.ap()[:, (n // 2) * 1024 : (n // 2 + 1) * 1024],
                        ots[n // 2][:],
                    )
    return nc


def _prepare(gene_set_features, flat_indices, segment_ids, segment_sizes):
    bounds, uidx_pads, s_sbs, T = _build_schedule(flat_indices, segment_ids)
    nc = _build_program(T)
    nc.compile()

    x = np.asarray(gene_set_features, dtype=np.float32)
    xt8 = np.ascontiguousarray(x.T).astype(F8)  # (G, B) e3m4
    sizes = np.asarray(segment_sizes, dtype=np.float32)
    gsz = _group_sizes(T)
    tstarts = np.cumsum([0] + gsz)

    in_maps = []
    for c in range(NCORES):
        lo_p, hi_p = bounds[c], bounds[c + 1]
        inv = np.ones((128, 1), np.float32)
        inv[: hi_p - lo_p, 0] = 1.0 / sizes[lo_p:hi_p]
        m = {"invsz": inv, "smat": s_sbs[c]}
        up = uidx_pads[c].reshape(T, 128)  # [t, p]
        for g, gs in enumerate(gsz):
            # slab row p holds the gs gene rows for partition p of group g,
            # concatenated: [ktile tstarts[g]+cc, partition p] for cc in gs.
            tiles = up[tstarts[g] : tstarts[g] + gs]        # [gs, 128]
            perm = tiles.T.reshape(-1)                       # [p, cc]
            m[f"slab{g}"] = np.ascontiguousarray(
                xt8[perm].reshape(128, gs * B)
            )
        in_maps.append(m)
    return nc, in_maps, bounds


def kernel(gene_set_features, flat_indices, segment_ids, segment_sizes, _res_hook=None):
    nc, in_maps, bounds = _prepare(
        gene_set_features, flat_indices, segment_ids, segment_sizes
    )
    res = run_bass_kernel_spmd(nc, in_maps, list(range(NCORES)))
    if _res_hook is not None:
        _res_hook(res)
    outT = np.empty((P, B), np.float32)
    for c in range(NCORES):
        lo_p, hi_p = bounds[c], bounds[c + 1]
        outT[lo_p:hi_p] = np.asarray(res.results[c]["out"]).astype(np.float32)[
            : hi_p - lo_p
        ]
    return np.ascontiguousarray(outT.T)


# revision 23
# speedup vs baseline: 1.1572x; 1.0055x over previous
"""Trainium2 Bass kernel for CellPathwayPoolingAggregator (segment mean).

out[b, p] = (1/segment_sizes[p]) * sum_{k: segment_ids[k]==p} x[b, flat_indices[k]]

Strategy (8 cores, sharded by contiguous pathway ranges):
  - Host: split the 1000 pathways into 8 contiguous ranges (<=128 pathways
    each) balancing per-core unique-gene counts. For each core, dedupe its
    gene rows and pack them into a contiguous DRAM slab in float8_e3m4
    (T k-tiles of 128 gene rows, grouped in GRP-tile DMA groups laid out so
    each DMA is a perfect 128-partition x (GRP*2KB)-per-partition contiguous
    transfer). A per-core count matrix S (e3m4, exact for small counts)
    carries the (gene, pathway) multiplicities.
  - Device (per core): plain sequential dma_start per group pulls the slab
    into SBUF; a PE matmul per (k-tile, 512-batch bank) accumulates
    pathway x batch sums into 4 PSUM banks (S tile stationary, gathered
    rows moving, fp32 accumulate). DVE/ACT scale rows by 1/segment_sizes,
    DMA stores the (128, 2048) f32 output slice; host reassembles.

e3m4 quantization of x gives rel err ~1.3e-2 (< 2e-2 tolerance); counts and
1/size scaling stay exact (counts are small ints; scale applied in f32).
"""

import sys

import numpy as np
import ml_dtypes

_TRN_REPO = "/opt/trn_rl_repo"
if _TRN_REPO not in sys.path:
    sys.path.insert(0, _TRN_REPO)

import concourse.bass as bass  # noqa: F401
import concourse.mybir as mybir
import concourse.tile as tile
from concourse import bacc
from concourse.bass_utils import run_bass_kernel_spmd

B, G, P = 2048, 10000, 1000
NCORES = 8
PC = 128          # max pathways per core (psum partition dim)
NB = B // 512     # matmul N-slices per K-tile (4 banks of 512 f32)
NWARM = 12        # PE warmup matmuls (ramp the tensor engine p-state)

F8 = ml_dtypes.float8_e3m4


def _group_sizes(T):
    """k-tiles per DMA group: uniform 2-tile (4KB/partition) groups.
    (A smaller leading group starves the PE at t=1 and resets its p-state
    ramp — measured worse.)"""
    gs = [2] * (T // 2)
    if T % 2:
        gs.append(1)
    return gs


def _split_ranges(seg_sorted, idx_sorted):
    """Contiguous pathway ranges, <=128 pathways each, minimizing the max
    per-core count of UNIQUE genes (which sets T and hence DMA/PE work)."""
    seg_starts = np.searchsorted(seg_sorted, np.arange(P + 1), side="left")

    def feasible(U):
        bounds = [0]
        for c in range(NCORES):
            lo_p = bounds[-1]
            if lo_p >= P:
                return None
            best = lo_p + 1
            hi_cap = min(P, lo_p + PC)
            lo_e = seg_starts[lo_p]
            for hi_p in range(lo_p + 1, hi_cap + 1):
                nu = len(np.unique(idx_sorted[lo_e : seg_starts[hi_p]]))
                if nu <= U:
                    best = hi_p
                else:
                    break
            bounds.append(best)
        return bounds if bounds[-1] >= P else None

    lo_t, hi_t = 1, (len(idx_sorted) + 127) // 128 + 1
    best_bounds = None
    while lo_t <= hi_t:
        mid = (lo_t + hi_t) // 2
        b = feasible(mid * 128)
        if b is not None:
            best_bounds = b
            hi_t = mid - 1
        else:
            lo_t = mid + 1
    if best_bounds is None:
        best_bounds = list(
            np.minimum(np.arange(NCORES + 1) * ((P + NCORES - 1) // NCORES), P)
        )
    best_bounds[-1] = P
    return best_bounds


def _build_schedule(flat_indices, segment_ids):
    seg = np.asarray(segment_ids, dtype=np.int64)
    idx = np.asarray(flat_indices, dtype=np.int64)
    order = np.argsort(seg, kind="stable")
    seg = seg[order]
    idx = idx[order]

    bounds = _split_ranges(seg, idx)
    cores = []
    for c in range(NCORES):
        lo_p, hi_p = bounds[c], bounds[c + 1]
        lo = np.searchsorted(seg, lo_p, side="left")
        hi = np.searchsorted(seg, hi_p, side="left")
        uidx, inv = np.unique(idx[lo:hi], return_inverse=True)
        cores.append((lo_p, hi_p, uidx, inv, seg[lo:hi] - lo_p))

    T = max(1, max((len(u) + 127) // 128 for _, _, u, _, _ in cores))
    T += T % 2  # even T: uniform 2-tile DMA groups (single-tag tile pool)
    Kpad = T * 128

    s_sbs, uidx_pads = [], []
    for lo_p, hi_p, uidx, inv, cols in cores:
        nu = len(uidx)
        # padded unique-gene list; pad rows point at gene 0 but S is zero there
        uidx_pad = np.concatenate([uidx, np.zeros(Kpad - nu, np.int64)])
        S = np.zeros((Kpad, PC), np.float32)
        np.add.at(S, (inv, cols), 1.0)
        S = S.astype(F8)
        s_sbs.append(
            np.ascontiguousarray(
                S.reshape(T, 128, PC).transpose(1, 0, 2).reshape(128, -1)
            )
        )
        uidx_pads.append(uidx_pad)
    return bounds, uidx_pads, s_sbs, T


def _build_program(T):
    nc = bacc.Bacc(
        "TRN2",
        target_bir_lowering=False,
        debug=False,
        num_devices=NCORES,
        num_swdge_queues=1,
    )
    f8, f32, bf16 = mybir.dt.float8e3, mybir.dt.float32, mybir.dt.bfloat16

    gsz = _group_sizes(T)
    NG = len(gsz)
    slab_ds = [
        nc.dram_tensor(f"slab{g}", [128, gsz[g] * B], f8, kind="ExternalInput")
        for g in range(NG)
    ]
    s_d = nc.dram_tensor("smat", [128, T * PC], f8, kind="ExternalInput")
    inv_d = nc.dram_tensor("invsz", [128, 1], f32, kind="ExternalInput")
    out_d = nc.dram_tensor("out", [PC, B], bf16, kind="ExternalOutput")

    with tile.TileContext(nc) as tc:
        with (
            tc.tile_pool(name="sb", bufs=1) as pool,
            tc.tile_pool(name="slabp", bufs=NG) as gpool,
            tc.tile_pool(name="psum", bufs=1, space="PSUM") as ppool,
        ):
            # Warmup source: memset on the (otherwise idle) Vector engine —
            # no DMA dependency, so the tensor engine starts ramping its
            # p-state immediately.
            wsrc = pool.tile([128, 512], f8, tag="wsrc")
            nc.vector.memset(wsrc[:], 0)

            # smat/invsz on the Scalar HWDGE queue, in parallel with the
            # slab groups on Sync. (GpSimd DMA is SWDGE — too slow here.)
            s_sb = pool.tile([128, T * PC], f8, tag="smat")
            nc.scalar.dma_start(s_sb[:], s_d.ap())
            inv_sb = pool.tile([128, 1], f32, tag="invsz")
            nc.scalar.dma_start(inv_sb[:], inv_d.ap())

            psb = [
                ppool.tile([128, 512], f32, tag=f"ps{n}", name=f"ps{n}")
                for n in range(NB)
            ]
            wps = ppool.tile([128, 512], f32, tag="pswarm", name="pswarm")

            gts = []
            for g in range(NG):
                gt = gpool.tile([128, gsz[g] * B], f8, tag="gt")
                nc.sync.dma_start(gt[:], slab_ds[g].ap())
                gts.append(gt)

            # Warmup matmuls: ramp the PE p-state while the first slab
            # group + smat are still in flight.
            for _ in range(NWARM):
                nc.tensor.matmul(
                    wps[:], wsrc[:, :128], wsrc[:], start=True, stop=True
                )

            tstarts = np.cumsum([0] + gsz)
            for g in range(NG):
                gt = gts[g]
                for cc in range(gsz[g]):
                    t = int(tstarts[g]) + cc
                    for n in range(NB):
                        nc.tensor.matmul(
                            psb[n][:],
                            s_sb[:, t * PC : (t + 1) * PC],
                            gt[:, cc * B + n * 512 : cc * B + (n + 1) * 512],
                            start=(t == 0),
                            stop=(t == T - 1),
                        )

            # Eviction: one engine per bank (DVE, ACT, DVE, GpSimd) into two
            # bf16 tiles; the two stores go out on separate HWDGE queues
            # (Sync and Scalar) so their issue+transfer overlap.
            ots = [
                pool.tile([128, 1024], bf16, tag=f"ot{i}", name=f"ot{i}")
                for i in range(2)
            ]
            for n in range(NB):
                ot = ots[n // 2][:, (n % 2) * 512 : (n % 2 + 1) * 512]
                if n == 1:
                    nc.scalar.activation(
                        ot,
                        psb[n][:],
                        mybir.ActivationFunctionType.Identity,
                        scale=inv_sb[:],
                    )
                else:
                    nc.vector.tensor_scalar_mul(ot, psb[n][:], inv_sb[:])
                if n % 2 == 1:
                    eng = nc.sync if n == 1 else nc.scalar
                    eng.dma_start(
                        out_d.ap()[:, (n // 2) * 1024 : (n // 2 + 1) * 1024],
                        ots[n // 2][:],
                    )
    return nc


def _prepare(gene_set_features, flat_indices, segment_ids, segment_sizes):
    bounds, uidx_pads, s_sbs, T = _build_schedule(flat_indices, segment_ids)
    nc = _build_program(T)
    nc.compile()

    x = np.asarray(gene_set_features, dtype=np.float32)
    xt8 = np.ascontiguousarray(x.T).astype(F8)  # (G, B) e3m4
    sizes = np.asarray(segment_sizes, dtype=np.float32)
    gsz = _group_sizes(T)
    tstarts = np.cumsum([0] + gsz)

    in_maps = []
    for c in range(NCORES):
        lo_p, hi_p = bounds[c], bounds[c + 1]
        inv = np.ones((128, 1), np.float32)
        inv[: hi_p - lo_p, 0] = 1.0 / sizes[lo_p:hi_p]
        m = {"invsz": inv, "smat": s_sbs[c]}
        up = uidx_pads[c].reshape(T, 128)  # [t, p]
        for g, gs in enumerate(gsz):
            # slab row p holds the gs gene rows for partition p of group g,
            # concatenated: [ktile tstarts[g]+cc, partition p] for cc in gs.
            tiles = up[tstarts[g] : tstarts[g] + gs]        # [gs, 128]
            perm = tiles.T.reshape(-1)                       # [p, cc]
            m[f"slab{g}"] = np.ascontiguousarray(
                xt8[perm].reshape(128, gs * B)
            )
        in_maps.append(m)
    return nc, in_maps, bounds


def kernel(gene_set_features, flat_indices, segment_ids, segment_sizes, _res_hook=None):
    nc, in_maps, bounds = _prepare(
        gene_set_features, flat_indices, segment_ids, segment_sizes
    )
    res = run_bass_kernel_spmd(nc, in_maps, list(range(NCORES)))
    if _res_hook is not None:
        _res_hook(res)
    outT = np.empty((P, B), np.float32)
    for c in range(NCORES):
        lo_p, hi_p = bounds[c], bounds[c + 1]
        outT[lo_p:hi_p] = np.asarray(res.results[c]["out"]).astype(np.float32)[
            : hi_p - lo_p
        ]
    return np.ascontiguousarray(outT.T)
